# revision 9
# baseline (speedup 1.0000x reference)
"""2-layer LSTM decoder Bass/Tile kernel for TRN2 — fp8 DoubleRow edition.

Per-core: B_local=128 batch rows, H=512, 64 steps. Data-parallel over 8
cores; weights replicated (host pre-transposes + pre-quantizes weights).

Layout "T": features on partitions, batch on the free dim (no transposes in
the recurrence). Numerics (validated vs reference in numpy sim, 3.4e-3):
  - i/f/o gate hidden/input-from-h matmuls: fp8 e4m3 DoubleRow (K=256/instr,
    0.5 cyc/row) with weights scaled x16; h state quantized to fp8 per step.
  - g (cell-candidate, tanh) gate matmuls: bf16 x bf16 (the tanh path
    dominates the error budget; sigmoid paths tolerate fp8 noise).
  - y feedback folded into the gate stream: in(t) = [y(t-1), x(t)] and
    y(t-1) = Wp h1(t-1) + bp, so the y contribution to L0 gates is
    W_feed @ h1(t-1) with W_feed = outer(Wih0[:,0], Wp) (fp8 DR stream) plus
    a constant row (bp + zb_y) carried in the K=10 input matmul. This takes
    proj->y->input off the critical path entirely.
  - L1 bias: single K=1 DoubleRow outer-product per chunk with (hi, lo) fp8
    rows against a fp8 ones vector (exact to ~6e-5).
  - PSUM is uniformly 16x; the gate ACTs apply scale=1/16. Tail elementwise
    in fp16 (DVE 2x mode); c state fp16; h written as bf16 (g-rhs + proj)
    and fp8 (i/f/o rhs) by two DVE ops.
"""

import numpy as np
from contextlib import ExitStack

import concourse.bass as bass
import concourse.bacc as bacc
import concourse.mybir as mybir
import concourse.tile as tile

F32 = mybir.dt.float32
F32R = mybir.dt.float32r
BF16 = mybir.dt.bfloat16
F16 = mybir.dt.float16
F8 = mybir.dt.float8e4
AF = mybir.ActivationFunctionType
OP = mybir.AluOpType
DRM = mybir.MatmulPerfMode.DoubleRow

P = 128           # batch rows per core
H = 512           # hidden
G = 2048          # 4*H gates
KC = 4            # K chunks of 128 across H
S = 64            # steps
EXO = 8
ZD = 16
SCL = 16.0        # weight scale baked into all gate-stream weights

# pytorch gate order by chunk: i: 0-3, f: 4-7, g: 8-11, o: 12-15
TYPE = "ifgo"  # chunk c//4 -> tile key
G_CHUNKS = (8, 9, 10, 11)

# emission order: g chunks first (tanh ACT fires early), then i, f, o
CH_ORDER = [8, 9, 10, 11, 0, 1, 2, 3, 4, 5, 6, 7, 12, 13, 14, 15]
# stop order: g first (tg early), o second (sg_o early), i+f last (the
# closing sigmoid covers both in one 1024-wide ACT)
STOP_GROUPS = [[8, 9, 10, 11], [12, 13, 14, 15], [0, 1, 2, 3, 4, 5, 6, 7]]
# start/stop are per 2KB PSUM bank: the "if" tile spans two banks, so both
# chunk 3 (i-bank) and chunk 7 (f-bank) carry stops
TILE_LAST = {11, 15, 3, 7}
NA1 = 12  # A-chunks emitted before the proj matmuls; o-chunks after

ROLES = {}  # instruction name -> role string (diagnostics only)
# ordered per-opcode role lists (diagnostics: trace slices pair up in order)
ORD = {"Matmult": [], "Activation": [], "TensorTensor": [], "TensorCopy": [],
       "Memset": []}


def _reg(kind, role):
    ORD[kind].append(role)


def _tag_role(inst, role):
    try:
        ROLES[inst.ins.name] = role
    except Exception:
        pass
    return inst


def build_kernel(nc: bass.Bass, steps: int):
    assert steps == S
    def di(name, shape, dt):
        return nc.dram_tensor(name, shape, dt, kind="ExternalInput").ap()

    # fp8 i/f/o weights, [p, k*G+g] = 16*W[g, k*128+p]
    w8h0 = di("w8h0", [P, KC * G], F8)
    w8i1 = di("w8i1", [P, KC * G], F8)
    w8h1 = di("w8h1", [P, KC * G], F8)
    wf8 = di("wf8", [P, KC * G], F8)          # 16*outer(Wih0[:,0], Wp[0])
    # bf16 g-gate weights, [p, k*512+j] = 16*W[1024+j, k*128+p]
    wg0 = di("wg0", [P, KC * H], BF16)
    wgi1 = di("wgi1", [P, KC * H], BF16)
    wgh1 = di("wgh1", [P, KC * H], BF16)
    # L0 input weights: rows 0-7 exo cols x16, row 8 = 16*b0, row 9 = 16*wcol
    wa0 = di("wa0", [10, G], BF16)
    b1p = di("b1p", [1, 2 * G], F8)           # (hi, lo) of 16*b1
    wpT = di("wpT", [P, KC], F32)             # [p, k] = W_proj[0, k*128+p]
    bp = di("bp", [1, 1], F32)
    wzT = di("wzT", [ZD, 9], F32)             # W_z.T
    bz8 = di("bz8", [8, 1], F32)              # b_z[1:9]
    bz0 = di("bz0", [1, 1], F32)              # b_z[0]
    ones_row = di("ones_row", [1, S * P], BF16)
    zT = di("zT", [ZD, P], F32)               # z.T
    y0T = di("y0T", [1, P], F32)
    xfT = di("xfT", [EXO, S * P], BF16)       # [e, t*128+b] = x_future[b, t, e]
    h0b_d = di("h0b", [P, H], BF16)           # [p, k*128+b] = h0[b, k*128+p]
    h1b_d = di("h1b", [P, H], BF16)
    h08_d = di("h08", [P, H], F8)
    h18_d = di("h18", [P, H], F8)
    c0_d = di("c0T", [P, H], F16)
    c1_d = di("c1T", [P, H], F16)
    out_d = nc.dram_tensor("out", [S, P], F32, kind="ExternalOutput").ap()

    with tile.TileContext(nc) as tc, ExitStack() as ctx:
        emit(ctx, tc, nc, locals())
    return nc


def emit(ctx, tc, nc, t_in):
    w8h0, w8i1, w8h1, wf8 = t_in["w8h0"], t_in["w8i1"], t_in["w8h1"], t_in["wf8"]
    wg0, wgi1, wgh1 = t_in["wg0"], t_in["wgi1"], t_in["wgh1"]
    wa0, b1p, wpT, bp = t_in["wa0"], t_in["b1p"], t_in["wpT"], t_in["bp"]
    wzT, bz8, bz0, zT = t_in["wzT"], t_in["bz8"], t_in["bz0"], t_in["zT"]
    ones_row, y0T, xfT = t_in["ones_row"], t_in["y0T"], t_in["xfT"]
    h0b_d, h1b_d, h08_d, h18_d = t_in["h0b_d"], t_in["h1b_d"], t_in["h08_d"], t_in["h18_d"]
    c0_d, c1_d, out_d = t_in["c0_d"], t_in["c1_d"], t_in["out_d"]

    # ---- pools ----
    const = ctx.enter_context(tc.tile_pool(name="const", bufs=1))
    ldtmp = ctx.enter_context(tc.tile_pool(name="ldtmp", bufs=1))
    state = ctx.enter_context(tc.tile_pool(name="state", bufs=2))
    act = ctx.enter_context(tc.tile_pool(name="act", bufs=2))
    yo = ctx.enter_context(tc.tile_pool(name="yo", bufs=3))
    g0p = ctx.enter_context(tc.tile_pool(name="g0p", bufs=1, space="PSUM"))
    g1p = ctx.enter_context(tc.tile_pool(name="g1p", bufs=1, space="PSUM"))

    dma = nc.sync.dma_start
    adma = nc.scalar.dma_start
    gdma = nc.gpsimd.dma_start

    # ---- persistent SBUF ----
    W8h0 = const.tile([P, KC * G], F8, name="W8h0")
    W8i1 = const.tile([P, KC * G], F8, name="W8i1")
    W8h1 = const.tile([P, KC * G], F8, name="W8h1")
    Wf8 = const.tile([P, KC * G], F8, name="Wf8")
    Wg0 = const.tile([P, KC * H], BF16, name="Wg0")
    Wgi1 = const.tile([P, KC * H], BF16, name="Wgi1")
    Wgh1 = const.tile([P, KC * H], BF16, name="Wgh1")
    wa0_sb = const.tile([10, G], BF16, name="wa0_sb")
    b1p_sb = const.tile([1, 2 * G], F8, name="b1p_sb")
    ones8 = const.tile([1, 2 * P], F8, name="ones8")
    wpT_sb = const.tile([P, KC], BF16, name="wpT_sb")
    inT = const.tile([10, S * P], BF16, name="inT")  # p0-7 exo, p8 ones, p9 feed
    zb8 = const.tile([8, P], BF16, name="zb8")     # z-bias for exo rows
    zby = const.tile([1, P], F32, name="zby")      # z-bias for the y slot
    bz8_sb = const.tile([8, 1], F32, name="bz8_sb")
    bz0_sb = const.tile([1, 1], F32, name="bz0_sb")
    bp_row = const.tile([1, P], F32, name="bp_row")

    # 3D views used by DR matmuls
    W8h0v = W8h0.rearrange("p (k g) -> p k g", k=KC)
    W8i1v = W8i1.rearrange("p (k g) -> p k g", k=KC)
    W8h1v = W8h1.rearrange("p (k g) -> p k g", k=KC)
    Wf8v = Wf8.rearrange("p (k g) -> p k g", k=KC)
    b1v = b1p_sb.rearrange("o (two g) -> o two g", two=2)
    on8v = ones8.rearrange("o (two b) -> o two b", two=2)

    # ---- init loads ----
    # SP queue: Wg0+W8h0 first (gate the step-0 A pass), then wa0
    dma(Wg0[:], wg0)
    dma(W8h0[:], w8h0)
    dma(wa0_sb[:], wa0)
    # gpsimd queue: z tensors (f32r cast loads), states, xfT, then L1/feed W
    wzT_sb = ldtmp.tile([ZD, 9], F32R, name="wzT_sb", tag="wz")
    gdma(wzT_sb[:], wzT)
    zT_sb = ldtmp.tile([ZD, P], F32R, name="zT_sb", tag="zt")
    gdma(zT_sb[:], zT)
    h0b_c = state.tile([P, H], BF16, name="h0b", tag="h0b")
    h08_c = state.tile([P, H], F8, name="h08", tag="h08")
    h1b_c = state.tile([P, H], BF16, name="h1b", tag="h1b")
    h18_c = state.tile([P, H], F8, name="h18", tag="h18")
    c0_c = state.tile([P, H], F16, name="c0", tag="c0")
    c1_c = state.tile([P, H], F16, name="c1", tag="c1")
    gdma(h0b_c[:], h0b_d)
    gdma(h08_c[:], h08_d)
    xfT_sb = ldtmp.tile([EXO, S * P], BF16, name="xfT_sb", tag="xf")
    gdma(xfT_sb[:], xfT)
    gdma(c0_c[:], c0_d)
    gdma(h1b_c[:], h1b_d)
    gdma(h18_c[:], h18_d)
    gdma(c1_c[:], c1_d)
    gdma(Wgh1[:], wgh1)
    gdma(W8h1[:], w8h1)
    gdma(Wgi1[:], wgi1)
    gdma(W8i1[:], w8i1)
    gdma(Wf8[:], wf8)
    # ACT queue: dummy sigmoid first so the table set loads immediately,
    # then the small admas that gate the zb ACTs
    dumm = ldtmp.tile([1, 1], F32, name="dumm", tag="dumm")
    _reg("Memset", "dumm")
    nc.vector.memset(dumm[:], 0.0)
    _reg("Activation", "dumm")
    nc.scalar.activation(dumm[:], dumm[:], AF.Sigmoid)
    adma(bz8_sb[:], bz8)
    adma(bz0_sb[:], bz0)
    y0T_sb = ldtmp.tile([1, P], F32, name="y0T_sb", tag="y0")
    adma(y0T_sb[:], y0T)
    adma(inT[8:9, :], ones_row)  # b0 ones row (host const)
    adma(b1p_sb[:], b1p)

    _reg("Memset", "ones8")
    nc.vector.memset(ones8[:], 1.0)

    # ---- z bias: zb8[8, P] = W_z[1:9] @ z.T + b_z[1:9]; zby = row 0 ----
    zb_ps = g0p.tile([P, H], F32, name="zbps", tag="g0if")
    _reg("Matmult", "zb")
    nc.tensor.matmul(
        zb_ps[0:8, 0:P], wzT_sb[:, 1:9], zT_sb[:],
        start=True, stop=True,
    )
    _reg("Matmult", "zb")
    nc.tensor.matmul(
        zb_ps[0:1, P : 2 * P], wzT_sb[:, 0:1], zT_sb[:],
        start=True, stop=True,
    )
    _reg("Activation", "zb8")
    nc.scalar.activation(zb8[:], zb_ps[0:8, 0:P], AF.Identity, bias=bz8_sb[:])
    _reg("Activation", "zby")
    nc.scalar.activation(zby[:], zb_ps[0:1, P : 2 * P], AF.Identity, bias=bz0_sb[:])

    # rest of the ACT-queue loads (after the zb ACTs so they don't gate them)
    bp_sb = ldtmp.tile([1, 1], F32, name="bp_sb", tag="bp")
    adma(bp_sb[:], bp)
    wpf = ldtmp.tile([P, KC], F32, name="wpf", tag="wp")
    adma(wpf[:], wpT)
    _reg("TensorCopy", "wpT")
    nc.vector.tensor_copy(wpT_sb[:], wpf[:])  # f32 -> bf16

    # exo rows with z-bias baked in: write straight into inT rows 0-7
    SPLIT = 4
    x3 = xfT_sb.rearrange("e (t b) -> e t b", b=P)
    dst3 = inT[0:8, :].rearrange("e (t b) -> e t b", b=P)
    zb3a = zb8.unsqueeze(1).broadcast_to((EXO, SPLIT, P))
    zb3b = zb8.unsqueeze(1).broadcast_to((EXO, S - SPLIT, P))
    _reg("TensorTensor", "exo_a")
    nc.vector.tensor_tensor(dst3[:, 0:SPLIT], x3[:, 0:SPLIT], zb3a, op=OP.add)
    # feed row (partition 9, engine-unwritable): staged base-0 then DMA'd.
    # block 0 = y0 + zby; blocks 1.. = bp + zby (y(t-1) = s(t-1) + bp and the
    # s part arrives via the W_feed matmuls).
    fr = ldtmp.tile([1, S * P], BF16, name="fr", tag="fr")
    _reg("TensorTensor", "fr0")
    nc.vector.tensor_tensor(fr[:, 0:P], y0T_sb[:], zby[:], op=OP.add)
    adma(inT[9:10, 0:P], fr[:, 0:P])
    _reg("TensorCopy", "bp_row")
    nc.vector.tensor_copy(bp_row[:], bp_sb[0:1, 0:1].broadcast_to((1, P)))
    zbybp = ldtmp.tile([1, P], BF16, name="zbybp", tag="zbybp")
    _reg("TensorTensor", "zbybp")
    nc.vector.tensor_tensor(zbybp[:], zby[:], bp_row[:], op=OP.add)
    fr3 = fr.rearrange("o (t b) -> o t b", b=P)
    zbb_a = zbybp.unsqueeze(1).broadcast_to((1, 8, P))
    _reg("TensorCopy", "fr_a")
    nc.vector.tensor_copy(fr3[:, 1:9], zbb_a)
    adma(inT[9:10, P : 9 * P], fr[:, P : 9 * P])
    # rest of the exo adds + feed row (slack-rich: only gates B(4+)/B(9+))
    _reg("TensorTensor", "exo_b")
    nc.vector.tensor_tensor(dst3[:, SPLIT:S], x3[:, SPLIT:S], zb3b, op=OP.add)
    zbb_b = zbybp.unsqueeze(1).broadcast_to((1, S - 9, P))
    _reg("TensorCopy", "fr_b")
    nc.vector.tensor_copy(fr3[:, 9:S], zbb_b)
    adma(inT[9:10, 9 * P :], fr[:, 9 * P :])

    # =========================== STEP LOOP ===========================
    CUR = ["init"]

    def R(role):
        CUR[0] = role

    def mm(*a, **k):
        _reg("Matmult", CUR[0])
        return _tag_role(nc.tensor.matmul(*a, **k), CUR[0])

    def sact(*a, **k):
        _reg("Activation", CUR[0])
        return _tag_role(nc.scalar.activation(*a, **k), CUR[0])

    def vtt(*a, **k):
        _reg("TensorTensor", CUR[0])
        return _tag_role(nc.vector.tensor_tensor(*a, **k), CUR[0])

    def g_tiles(pool, pfx):
        return {
            "if": pool.tile([P, 2 * H], F32, name=pfx + "if", tag=pfx + "if"),
            "g": pool.tile([P, H], F32, name=pfx + "g", tag=pfx + "g"),
            "o": pool.tile([P, H], F32, name=pfx + "o", tag=pfx + "o"),
        }

    def tkey(c):
        return "if" if c < 8 else ("g" if c < 12 else "o")

    def bkey(c):
        # PSUM zero-region bookkeeping key: one per 2KB bank
        return f"if{c // 4}" if c < 8 else tkey(c)

    def dst_of(g, c):
        if c < 8:
            return g["if"][:, c * P : (c + 1) * P]
        return g[tkey(c)][:, (c % 4) * P : (c % 4 + 1) * P]

    def hid_mms(g, c, W8v, Wg, h8v, hb, started, stop=False):
        """Hidden-side matmuls for one gate chunk: fp8 DR for i/f/o, bf16
        for g. One PSUM start per tile (zero region covers the bank)."""
        dst = dst_of(g, c)
        ty = bkey(c)
        if c in G_CHUNKS:
            j0 = (c - 8) * P
            for k in range(KC):
                mm(
                    dst, Wg[:, k * H + j0 : k * H + j0 + P],
                    hb[:, k * P : (k + 1) * P],
                    start=not started.get(ty, False),
                    stop=(stop and k == KC - 1),
                )
                started[ty] = True
        else:
            for k2 in range(2):
                mm(
                    dst, W8v[:, 2 * k2 : 2 * k2 + 2, c * P : (c + 1) * P],
                    h8v[:, 2 * k2 : 2 * k2 + 2, :],
                    start=not started.get(ty, False),
                    stop=(stop and k2 == 1),
                    perf_mode=DRM,
                )
                started[ty] = True

    def emit_A(g0, h08, h0b, chunks, started):
        """L0 hidden matmuls (start-side of each tile's group)."""
        R("A")
        h8v = h08.rearrange("p (k b) -> p k b", k=KC)
        for c in chunks:
            hid_mms(g0, c, W8h0v, Wg0, h8v, h0b, started)

    def emit_B(g0, t, parts=None):
        """L0 input matmuls, K=10 bf16 (exo + b0-ones + feed-const row).
        With parts: carries the group stops (t=0 only)."""
        rhs = inT[:, t * P : (t + 1) * P]
        if parts is None:
            R("B")
            for c in CH_ORDER:
                mm(dst_of(g0, c), wa0_sb[:, c * P : (c + 1) * P], rhs,
                   start=False, stop=False)
        else:
            for grp, part in zip(STOP_GROUPS, parts):
                R("B")
                for c in grp:
                    mm(dst_of(g0, c), wa0_sb[:, c * P : (c + 1) * P], rhs,
                       start=False, stop=(c in TILE_LAST))
                part()

    def emit_feed(g0, h18, parts):
        """W_feed @ h1 (fp8 DR, all 16 chunks): the tile-closing stops;
        tail parts interleave between stop groups."""
        h8v = h18.rearrange("p (k b) -> p k b", k=KC)
        for grp, part in zip(STOP_GROUPS, parts):
            R("feed")
            for c in grp:
                dst = dst_of(g0, c)
                for k2 in range(2):
                    mm(
                        dst, Wf8v[:, 2 * k2 : 2 * k2 + 2, c * P : (c + 1) * P],
                        h8v[:, 2 * k2 : 2 * k2 + 2, :],
                        start=False, stop=(c in TILE_LAST and k2 == 1),
                        perf_mode=DRM,
                    )
            part()

    def emit_C(g1, h18, h1b, chunks, started):
        """L1 h1-part matmuls + fp8 DR bias (start-side)."""
        h8v = h18.rearrange("p (k b) -> p k b", k=KC)
        for c in chunks:
            R("C")
            hid_mms(g1, c, W8h1v, Wgh1, h8v, h1b, started)
            R("Cb")
            mm(
                dst_of(g1, c), b1v[:, :, c * P : (c + 1) * P], on8v[:],
                start=False, stop=False, perf_mode=DRM,
            )

    def emit_D(g1, h08, h0b, parts):
        """L1 h0-part matmuls: the tile-closing stops; tail parts
        interleave between stop groups."""
        h8v = h08.rearrange("p (k b) -> p k b", k=KC)
        for grp, part in zip(STOP_GROUPS, parts):
            R("D")
            for c in grp:
                hid_mms(g1, c, W8i1v, Wgi1, h8v, h0b, {bkey(c): True},
                        stop=(c in TILE_LAST))
            part()

    def make_tail(g, c_prev, htag, ctag, h8_first):
        """Tail for one layer as 3 emit-callbacks matching stop groups
        (g, o, if). ACTs full-width; the DVE c-chain and tanh(c)/h-writes
        in halves so downstream consumers start early. h8_first orders the
        fp8 h write before the bf16 one (L1: feed needs h18; L0: D-g needs
        h0 bf16 first)."""
        sg_if = act.tile([P, 2 * H], F16, name="sg_if", tag="sg_if")
        tg = act.tile([P, H], F16, name="tg", tag="tg")
        sg_o = act.tile([P, H], F16, name="sg_o", tag="sg_o")
        t1 = act.tile([P, H], F16, name="t1", tag="t1")
        t2 = act.tile([P, H], F16, name="t2", tag="t2")
        thc = act.tile([P, H], F16, name="thc", tag="thc")
        c_n = state.tile([P, H], F16, name=ctag, tag=ctag)
        h_b = state.tile([P, H], BF16, name=htag + "b", tag=htag + "b")
        h_8 = state.tile([P, H], F8, name=htag + "8", tag=htag + "8")
        sg_i, sg_f = sg_if[:, 0:H], sg_if[:, H : 2 * H]

        ISC = 1.0 / SCL
        HH = H // 2
        ha, hb = slice(0, HH), slice(HH, H)

        def p_g():
            R(htag + ":tg")
            sact(tg[:], g["g"][:], AF.Tanh, scale=ISC)

        def p_o():
            R(htag + ":sg_o")
            sact(sg_o[:], g["o"][:], AF.Sigmoid, scale=ISC)

        def p_if():
            R(htag + ":sg_if")
            sact(sg_if[:], g["if"][:], AF.Sigmoid, scale=ISC)
            R(htag + ":cchain")
            vtt(t2[:, ha], sg_f[:, ha], c_prev[:, ha], op=OP.mult)
            vtt(t1[:, ha], sg_i[:, ha], tg[:, ha], op=OP.mult)
            vtt(c_n[:, ha], t1[:, ha], t2[:, ha], op=OP.add)
            sact(thc[:, ha], c_n[:, ha], AF.Tanh)
            vtt(t2[:, hb], sg_f[:, hb], c_prev[:, hb], op=OP.mult)
            vtt(t1[:, hb], sg_i[:, hb], tg[:, hb], op=OP.mult)
            vtt(c_n[:, hb], t1[:, hb], t2[:, hb], op=OP.add)
            sact(thc[:, hb], c_n[:, hb], AF.Tanh)
            R(htag + ":hwr")
            outs = [h_8, h_b] if h8_first else [h_b, h_8]
            for dst in outs:
                vtt(dst[:, ha], sg_o[:, ha], thc[:, ha], op=OP.mult)
                vtt(dst[:, hb], sg_o[:, hb], thc[:, hb], op=OP.mult)

        return [p_g, p_o, p_if], c_n, h_b, h_8

    # prologue: step-0 L0 gates (+interleaved tail via B stops) and L1 start
    g0_cur = g_tiles(g0p, "g0")
    emit_A(g0_cur, h08_c, h0b_c, CH_ORDER, {})
    parts0, c0_n, h0b_cur, h08_cur = make_tail(g0_cur, c0_c, "h0", "c0", h8_first=False)
    emit_B(g0_cur, 0, parts0)
    c0_c = c0_n
    g1_cur = g_tiles(g1p, "g1")
    emit_C(g1_cur, h18_c, h1b_c, CH_ORDER, {})

    for t in range(S):
        # ---- PE: L1 h0-part for t, interleaved with the L1 tail ----
        parts1, c1_n, h1b_n, h18_n = make_tail(g1_cur, c1_c, "h1", "c1", h8_first=True)
        emit_D(g1_cur, h08_cur, h0b_cur, parts1)
        c1_c = c1_n
        # ---- PE: L0 gates for t+1, first part (covers L1-tail latency) ----
        if t + 1 < S:
            g0_nxt = g_tiles(g0p, "g0")
            a_started = {}
            emit_A(g0_nxt, h08_cur, h0b_cur, CH_ORDER[:NA1], a_started)
        # ---- PE: projection yT(t) = Wp @ h1(t) (output only, off-path) ----
        pj = g1_cur["o"][0:1, 3 * P : 4 * P]
        R("proj")
        for k in range(2):
            mm(pj, wpT_sb[:, k : k + 1], h1b_n[:, k * P : (k + 1) * P],
               start=(k == 0), stop=False)
        if t + 1 < S:
            emit_A(g0_nxt, h08_cur, h0b_cur, CH_ORDER[NA1 : NA1 + 1], a_started)
        R("proj")
        for k in range(2, KC):
            mm(pj, wpT_sb[:, k : k + 1], h1b_n[:, k * P : (k + 1) * P],
               start=False, stop=(k == KC - 1))
        yo_t = yo.tile([1, P], F32, name="yout", tag="yout")
        R("y_out")
        vtt(yo_t[:], pj, bp_row[:], op=OP.add)
        dma(out_d[t : t + 1, :], yo_t[:])
        if t + 1 < S:
            emit_A(g0_nxt, h08_cur, h0b_cur, CH_ORDER[NA1 + 1 :], a_started)
            emit_B(g0_nxt, t + 1)
            parts0, c0_n, h0b_nxt, h08_nxt = make_tail(g0_nxt, c0_c, "h0", "c0", h8_first=False)
            emit_feed(g0_nxt, h18_n, parts0)
            c0_c, h0b_cur, h08_cur = c0_n, h0b_nxt, h08_nxt
            g1_nxt = g_tiles(g1p, "g1")
            emit_C(g1_nxt, h18_n, h1b_n, CH_ORDER, {})
            g0_cur, g1_cur = g0_nxt, g1_nxt


def make_nc(steps: int = S):
    nc = bacc.Bacc("TRN2", target_bir_lowering=False, debug=False)
    build_kernel(nc, steps)
    nc.compile()
    return nc


# ======================= host-side prep =======================

def _bf16(x):
    import ml_dtypes
    return np.ascontiguousarray(np.asarray(x, np.float32).astype(ml_dtypes.bfloat16))


def _f8(x):
    import ml_dtypes
    return np.ascontiguousarray(np.asarray(x, np.float32).astype(ml_dtypes.float8_e4m3fn))


def _f32(x):
    return np.ascontiguousarray(np.asarray(x, dtype=np.float32))


def _wT(W):
    """[G', H] f32 -> [128, KC*G'] with [p, k*G'+g] = W[g, k*128+p]."""
    W = np.asarray(W, dtype=np.float32)
    Gp = W.shape[0]
    return W.T.reshape(KC, P, Gp).transpose(1, 0, 2).reshape(P, KC * Gp)


def _hT4(h):
    """[B_loc, H] -> [128, H] with [p, k*128+b] = h[b, k*128+p]."""
    return h.T.reshape(KC, P, P).transpose(1, 0, 2).reshape(P, H)


def shard_inputs(inputs, steps: int):
    import ml_dtypes
    B = inputs["y0"].shape[0]
    nb = B // P
    Wih0 = np.asarray(inputs["W_ih0"], np.float32)
    Wp = np.asarray(inputs["W_proj"], np.float32)
    b0 = _f32(inputs["b_ih0"]) + _f32(inputs["b_hh0"])
    b1 = (_f32(inputs["b_ih1"]) + _f32(inputs["b_hh1"])) * SCL
    b1hi = b1.astype(ml_dtypes.float8_e4m3fn)
    b1lo = (b1 - b1hi.astype(np.float32)).astype(ml_dtypes.float8_e4m3fn)
    wa0 = np.concatenate(
        [
            Wih0[:, 1:9].T * SCL,          # exo rows
            b0.reshape(1, G) * SCL,        # ones-row bias
            Wih0[:, 0:1].T * SCL,          # feed-const row (wcol)
        ],
        axis=0,
    )
    Wf = np.outer(Wih0[:, 0], Wp[0])       # [G, H] y-feedback weights

    def wsplit(W):
        W = np.asarray(W, np.float32) * SCL
        return _f8(_wT(W)), _bf16(_wT(W[1024:1536]))

    w8h0, wg0 = wsplit(inputs["W_hh0"])
    w8i1, wgi1 = wsplit(inputs["W_ih1"])
    w8h1, wgh1 = wsplit(inputs["W_hh1"])
    rep = {
        "w8h0": w8h0, "wg0": wg0,
        "w8i1": w8i1, "wgi1": wgi1,
        "w8h1": w8h1, "wgh1": wgh1,
        "wf8": _f8(_wT(Wf * SCL)),
        "wa0": _bf16(wa0),
        "b1p": np.ascontiguousarray(
            np.stack([b1hi, b1lo]).reshape(1, 2 * G)),
        "wpT": _f32(Wp).reshape(KC, P).T.copy(),
        "bp": _f32(inputs["b_proj"]).reshape(1, 1),
        "wzT": _f32(inputs["W_z"]).T.copy(),
        "bz8": _f32(inputs["b_z"]).reshape(9, 1)[1:9].copy(),
        "bz0": _f32(inputs["b_z"]).reshape(9, 1)[0:1].copy(),
        "ones_row": _bf16(np.ones((1, S * P), np.float32)),
    }
    maps = []
    for i in range(nb):
        s = slice(i * P, (i + 1) * P)
        x = np.asarray(inputs["x_future"], np.float32)[s, :steps]  # [P, S, E]
        h0b = _bf16(_hT4(np.asarray(inputs["h0"], np.float32)[0, s]))
        h1b = _bf16(_hT4(np.asarray(inputs["h0"], np.float32)[1, s]))
        m = dict(rep)
        m.update(
            {
                "zT": _f32(inputs["z"][s]).T.copy(),
                "y0T": _f32(inputs["y0"][s]).reshape(1, P).copy(),
                "xfT": _bf16(x.transpose(2, 1, 0).reshape(EXO, steps * P)),
                "h0b": h0b,
                "h1b": h1b,
                "h08": _f8(h0b.astype(np.float32)),
                "h18": _f8(h1b.astype(np.float32)),
                "c0T": np.ascontiguousarray(
                    _hT4(np.asarray(inputs["c0"], np.float32)[0, s]).astype(np.float16)),
                "c1T": np.ascontiguousarray(
                    _hT4(np.asarray(inputs["c0"], np.float32)[1, s]).astype(np.float16)),
            }
        )
        maps.append(m)
    return maps


def assemble_output(results, steps: int):
    outs = [
        np.ascontiguousarray(np.asarray(rm["out"]).T).reshape(P, steps, 1)
        for rm in results
    ]
    return np.concatenate(outs, axis=0)


# ======================= public entry point =======================
_NC_CACHE = {}


def _get_nc():
    if "nc" not in _NC_CACHE:
        _NC_CACHE["nc"] = make_nc(S)
    return _NC_CACHE["nc"]


STEPS = S
N_CORES = 8


def kernel(**inputs):
    """Full-input entry point: shards batch over 8 NeuronCores, runs the
    Bass LSTM-decoder kernel, reassembles [B, steps, 1] float32 output."""
    from concourse.bass_utils import run_bass_kernel_spmd

    steps = int(inputs.get("steps", STEPS))
    assert steps == STEPS, f"kernel compiled for {STEPS} steps, got {steps}"
    nc = _get_nc()
    maps = shard_inputs(inputs, STEPS)
    res = run_bass_kernel_spmd(nc, maps, list(range(N_CORES)))
    return assemble_output(res.results, STEPS).astype(np.float32)


# revision 10
# speedup vs baseline: 1.0162x; 1.0162x over previous
"""2-layer LSTM decoder Bass/Tile kernel for TRN2 — fp8 DoubleRow edition.

Per-core: B_local=128 batch rows, H=512, 64 steps. Data-parallel over 8
cores; weights replicated (host pre-transposes + pre-quantizes weights).

Layout "T": features on partitions, batch on the free dim (no transposes in
the recurrence). Numerics (validated vs reference in numpy sim, 3.4e-3):
  - i/f/o gate hidden/input-from-h matmuls: fp8 e4m3 DoubleRow (K=256/instr,
    0.5 cyc/row) with weights scaled x16; h state quantized to fp8 per step.
  - g (cell-candidate, tanh) gate matmuls: bf16 x bf16 (the tanh path
    dominates the error budget; sigmoid paths tolerate fp8 noise).
  - y feedback folded into the gate stream: in(t) = [y(t-1), x(t)] and
    y(t-1) = Wp h1(t-1) + bp, so the y contribution to L0 gates is
    W_feed @ h1(t-1) with W_feed = outer(Wih0[:,0], Wp) (fp8 DR stream) plus
    a constant row (bp + zb_y) carried in the K=10 input matmul. This takes
    proj->y->input off the critical path entirely.
  - L1 bias: single K=1 DoubleRow outer-product per chunk with (hi, lo) fp8
    rows against a fp8 ones vector (exact to ~6e-5).
  - PSUM is uniformly 16x; the gate ACTs apply scale=1/16. Tail elementwise
    in fp16 (DVE 2x mode); c state fp16; h written as bf16 (g-rhs + proj)
    and fp8 (i/f/o rhs) by two DVE ops.
"""

import numpy as np
from contextlib import ExitStack

import concourse.bass as bass
import concourse.bacc as bacc
import concourse.mybir as mybir
import concourse.tile as tile

F32 = mybir.dt.float32
F32R = mybir.dt.float32r
BF16 = mybir.dt.bfloat16
F16 = mybir.dt.float16
F8 = mybir.dt.float8e4
AF = mybir.ActivationFunctionType
OP = mybir.AluOpType
DRM = mybir.MatmulPerfMode.DoubleRow

P = 128           # batch rows per core
H = 512           # hidden
G = 2048          # 4*H gates
KC = 4            # K chunks of 128 across H
S = 64            # steps
EXO = 8
ZD = 16
SCL = 16.0        # weight scale baked into all gate-stream weights

# pytorch gate order by chunk: i: 0-3, f: 4-7, g: 8-11, o: 12-15
TYPE = "ifgo"  # chunk c//4 -> tile key
G_CHUNKS = (8, 9, 10, 11)

# emission order: g chunks first (tanh ACT fires early), then i, f, o
CH_ORDER = [8, 9, 10, 11, 0, 1, 2, 3, 4, 5, 6, 7, 12, 13, 14, 15]
# stop order: g first (tg fires early), then i+f+o (ONE 1536-wide closing
# sigmoid covers all three)
STOP_GROUPS = [[8, 9, 10, 11], [0, 1, 2, 3, 4, 5, 6, 7, 12, 13, 14, 15]]
# start/stop are per 2KB PSUM bank: the "ifo" tile spans three banks, so
# chunks 3 (i-bank), 7 (f-bank) and 15 (o-bank) all carry stops
TILE_LAST = {11, 3, 7, 15}
NA1 = 12  # A-chunks emitted before the proj matmuls; o-chunks after

ROLES = {}  # instruction name -> role string (diagnostics only)
# ordered per-opcode role lists (diagnostics: trace slices pair up in order)
ORD = {"Matmult": [], "Activation": [], "TensorTensor": [], "TensorCopy": [],
       "Memset": []}


def _reg(kind, role):
    ORD[kind].append(role)


def _tag_role(inst, role):
    try:
        ROLES[inst.ins.name] = role
    except Exception:
        pass
    return inst


def build_kernel(nc: bass.Bass, steps: int):
    assert steps == S
    def di(name, shape, dt):
        return nc.dram_tensor(name, shape, dt, kind="ExternalInput").ap()

    # fp8 i/f/o weights, [p, k*G+g] = 16*W[g, k*128+p]
    w8h0 = di("w8h0", [P, KC * G], F8)
    w8i1 = di("w8i1", [P, KC * G], F8)
    w8h1 = di("w8h1", [P, KC * G], F8)
    wf8 = di("wf8", [P, KC * G], F8)          # 16*outer(Wih0[:,0], Wp[0])
    # bf16 g-gate weights, [p, k*512+j] = 16*W[1024+j, k*128+p]
    wg0 = di("wg0", [P, KC * H], BF16)
    wgi1 = di("wgi1", [P, KC * H], BF16)
    wgh1 = di("wgh1", [P, KC * H], BF16)
    # L0 input weights: rows 0-7 exo cols x16, row 8 = 16*b0, row 9 = 16*wcol
    wa0 = di("wa0", [10, G], BF16)
    b1p = di("b1p", [1, 2 * G], F8)           # (hi, lo) of 16*b1
    wpT = di("wpT", [P, KC], F32)             # [p, k] = W_proj[0, k*128+p]
    bp = di("bp", [1, 1], F32)
    wzT = di("wzT", [ZD, 9], F32)             # W_z.T
    bz8 = di("bz8", [8, 1], F32)              # b_z[1:9]
    bz0 = di("bz0", [1, 1], F32)              # b_z[0]
    ones_row = di("ones_row", [1, S * P], BF16)
    zT = di("zT", [ZD, P], F32)               # z.T
    y0T = di("y0T", [1, P], F32)
    xfT = di("xfT", [EXO, S * P], BF16)       # [e, t*128+b] = x_future[b, t, e]
    h0b_d = di("h0b", [P, H], BF16)           # [p, k*128+b] = h0[b, k*128+p]
    h1b_d = di("h1b", [P, H], BF16)
    h08_d = di("h08", [P, H], F8)
    h18_d = di("h18", [P, H], F8)
    c0_d = di("c0T", [P, H], F16)
    c1_d = di("c1T", [P, H], F16)
    out_d = nc.dram_tensor("out", [S, P], F32, kind="ExternalOutput").ap()

    with tile.TileContext(nc) as tc, ExitStack() as ctx:
        emit(ctx, tc, nc, locals())
    return nc


def emit(ctx, tc, nc, t_in):
    w8h0, w8i1, w8h1, wf8 = t_in["w8h0"], t_in["w8i1"], t_in["w8h1"], t_in["wf8"]
    wg0, wgi1, wgh1 = t_in["wg0"], t_in["wgi1"], t_in["wgh1"]
    wa0, b1p, wpT, bp = t_in["wa0"], t_in["b1p"], t_in["wpT"], t_in["bp"]
    wzT, bz8, bz0, zT = t_in["wzT"], t_in["bz8"], t_in["bz0"], t_in["zT"]
    ones_row, y0T, xfT = t_in["ones_row"], t_in["y0T"], t_in["xfT"]
    h0b_d, h1b_d, h08_d, h18_d = t_in["h0b_d"], t_in["h1b_d"], t_in["h08_d"], t_in["h18_d"]
    c0_d, c1_d, out_d = t_in["c0_d"], t_in["c1_d"], t_in["out_d"]

    # ---- pools ----
    const = ctx.enter_context(tc.tile_pool(name="const", bufs=1))
    ldtmp = ctx.enter_context(tc.tile_pool(name="ldtmp", bufs=1))
    state = ctx.enter_context(tc.tile_pool(name="state", bufs=2))
    act = ctx.enter_context(tc.tile_pool(name="act", bufs=2))
    yo = ctx.enter_context(tc.tile_pool(name="yo", bufs=3))
    g0p = ctx.enter_context(tc.tile_pool(name="g0p", bufs=1, space="PSUM"))
    g1p = ctx.enter_context(tc.tile_pool(name="g1p", bufs=1, space="PSUM"))

    dma = nc.sync.dma_start
    adma = nc.scalar.dma_start
    gdma = nc.gpsimd.dma_start

    # ---- persistent SBUF ----
    W8h0 = const.tile([P, KC * G], F8, name="W8h0")
    W8i1 = const.tile([P, KC * G], F8, name="W8i1")
    W8h1 = const.tile([P, KC * G], F8, name="W8h1")
    Wf8 = const.tile([P, KC * G], F8, name="Wf8")
    Wg0 = const.tile([P, KC * H], BF16, name="Wg0")
    Wgi1 = const.tile([P, KC * H], BF16, name="Wgi1")
    Wgh1 = const.tile([P, KC * H], BF16, name="Wgh1")
    wa0_sb = const.tile([10, G], BF16, name="wa0_sb")
    b1p_sb = const.tile([1, 2 * G], F8, name="b1p_sb")
    ones8 = const.tile([1, 2 * P], F8, name="ones8")
    wpT_sb = const.tile([P, KC], BF16, name="wpT_sb")
    inT = const.tile([10, S * P], BF16, name="inT")  # p0-7 exo, p8 ones, p9 feed
    zb8 = const.tile([8, P], BF16, name="zb8")     # z-bias for exo rows
    zby = const.tile([1, P], F32, name="zby")      # z-bias for the y slot
    bz8_sb = const.tile([8, 1], F32, name="bz8_sb")
    bz0_sb = const.tile([1, 1], F32, name="bz0_sb")
    bp_row = const.tile([1, P], F32, name="bp_row")

    # 3D views used by DR matmuls
    W8h0v = W8h0.rearrange("p (k g) -> p k g", k=KC)
    W8i1v = W8i1.rearrange("p (k g) -> p k g", k=KC)
    W8h1v = W8h1.rearrange("p (k g) -> p k g", k=KC)
    Wf8v = Wf8.rearrange("p (k g) -> p k g", k=KC)
    b1v = b1p_sb.rearrange("o (two g) -> o two g", two=2)
    on8v = ones8.rearrange("o (two b) -> o two b", two=2)

    # ---- init loads ----
    # SP queue: Wg0+W8h0 first (gate the step-0 A pass), then wa0
    dma(Wg0[:], wg0)
    dma(W8h0[:], w8h0)
    dma(wa0_sb[:], wa0)
    # gpsimd queue: z tensors (f32r cast loads), states, xfT, then L1/feed W
    wzT_sb = ldtmp.tile([ZD, 9], F32R, name="wzT_sb", tag="wz")
    gdma(wzT_sb[:], wzT)
    zT_sb = ldtmp.tile([ZD, P], F32R, name="zT_sb", tag="zt")
    gdma(zT_sb[:], zT)
    h0b_c = state.tile([P, H], BF16, name="h0b", tag="h0b")
    h08_c = state.tile([P, H], F8, name="h08", tag="h08")
    h1b_c = state.tile([P, H], BF16, name="h1b", tag="h1b")
    h18_c = state.tile([P, H], F8, name="h18", tag="h18")
    c0_c = state.tile([P, H], F16, name="c0", tag="c0")
    c1_c = state.tile([P, H], F16, name="c1", tag="c1")
    gdma(h0b_c[:], h0b_d)
    gdma(h08_c[:], h08_d)
    xfT_sb = ldtmp.tile([EXO, S * P], BF16, name="xfT_sb", tag="xf")
    gdma(xfT_sb[:], xfT)
    gdma(c0_c[:], c0_d)
    gdma(h1b_c[:], h1b_d)
    gdma(h18_c[:], h18_d)
    gdma(c1_c[:], c1_d)
    gdma(Wgh1[:], wgh1)
    gdma(W8h1[:], w8h1)
    gdma(Wgi1[:], wgi1)
    gdma(W8i1[:], w8i1)
    gdma(Wf8[:], wf8)
    # ACT queue: dummy sigmoid first so the table set loads immediately,
    # then the small admas that gate the zb ACTs
    dumm = ldtmp.tile([1, 1], F32, name="dumm", tag="dumm")
    _reg("Memset", "dumm")
    nc.vector.memset(dumm[:], 0.0)
    _reg("Activation", "dumm")
    nc.scalar.activation(dumm[:], dumm[:], AF.Sigmoid)
    adma(bz8_sb[:], bz8)
    adma(bz0_sb[:], bz0)
    y0T_sb = ldtmp.tile([1, P], F32, name="y0T_sb", tag="y0")
    adma(y0T_sb[:], y0T)
    adma(inT[8:9, :], ones_row)  # b0 ones row (host const)
    adma(b1p_sb[:], b1p)

    _reg("Memset", "ones8")
    nc.vector.memset(ones8[:], 1.0)

    # ---- z bias: zb8[8, P] = W_z[1:9] @ z.T + b_z[1:9]; zby = row 0 ----
    zb_ps = g0p.tile([P, H], F32, name="zbps", tag="g0ifo")
    _reg("Matmult", "zb")
    nc.tensor.matmul(
        zb_ps[0:8, 0:P], wzT_sb[:, 1:9], zT_sb[:],
        start=True, stop=True,
    )
    _reg("Matmult", "zb")
    nc.tensor.matmul(
        zb_ps[0:1, P : 2 * P], wzT_sb[:, 0:1], zT_sb[:],
        start=True, stop=True,
    )
    _reg("Activation", "zb8")
    nc.scalar.activation(zb8[:], zb_ps[0:8, 0:P], AF.Identity, bias=bz8_sb[:])
    _reg("Activation", "zby")
    nc.scalar.activation(zby[:], zb_ps[0:1, P : 2 * P], AF.Identity, bias=bz0_sb[:])

    # rest of the ACT-queue loads (after the zb ACTs so they don't gate them)
    bp_sb = ldtmp.tile([1, 1], F32, name="bp_sb", tag="bp")
    adma(bp_sb[:], bp)
    wpf = ldtmp.tile([P, KC], F32, name="wpf", tag="wp")
    adma(wpf[:], wpT)
    _reg("TensorCopy", "wpT")
    nc.vector.tensor_copy(wpT_sb[:], wpf[:])  # f32 -> bf16

    # exo rows with z-bias baked in: write straight into inT rows 0-7
    SPLIT = 4
    x3 = xfT_sb.rearrange("e (t b) -> e t b", b=P)
    dst3 = inT[0:8, :].rearrange("e (t b) -> e t b", b=P)
    zb3a = zb8.unsqueeze(1).broadcast_to((EXO, SPLIT, P))
    zb3b = zb8.unsqueeze(1).broadcast_to((EXO, S - SPLIT, P))
    _reg("TensorTensor", "exo_a")
    nc.vector.tensor_tensor(dst3[:, 0:SPLIT], x3[:, 0:SPLIT], zb3a, op=OP.add)
    # feed row (partition 9, engine-unwritable): staged base-0 then DMA'd.
    # block 0 = y0 + zby; blocks 1.. = bp + zby (y(t-1) = s(t-1) + bp and the
    # s part arrives via the W_feed matmuls).
    fr = ldtmp.tile([1, S * P], BF16, name="fr", tag="fr")
    _reg("TensorTensor", "fr0")
    nc.vector.tensor_tensor(fr[:, 0:P], y0T_sb[:], zby[:], op=OP.add)
    adma(inT[9:10, 0:P], fr[:, 0:P])
    _reg("TensorCopy", "bp_row")
    nc.vector.tensor_copy(bp_row[:], bp_sb[0:1, 0:1].broadcast_to((1, P)))
    zbybp = ldtmp.tile([1, P], BF16, name="zbybp", tag="zbybp")
    _reg("TensorTensor", "zbybp")
    nc.vector.tensor_tensor(zbybp[:], zby[:], bp_row[:], op=OP.add)
    fr3 = fr.rearrange("o (t b) -> o t b", b=P)
    zbb_a = zbybp.unsqueeze(1).broadcast_to((1, 8, P))
    _reg("TensorCopy", "fr_a")
    nc.vector.tensor_copy(fr3[:, 1:9], zbb_a)
    adma(inT[9:10, P : 9 * P], fr[:, P : 9 * P])
    # rest of the exo adds + feed row (slack-rich: only gates B(4+)/B(9+))
    _reg("TensorTensor", "exo_b")
    nc.vector.tensor_tensor(dst3[:, SPLIT:S], x3[:, SPLIT:S], zb3b, op=OP.add)
    zbb_b = zbybp.unsqueeze(1).broadcast_to((1, S - 9, P))
    _reg("TensorCopy", "fr_b")
    nc.vector.tensor_copy(fr3[:, 9:S], zbb_b)
    adma(inT[9:10, 9 * P :], fr[:, 9 * P :])

    # =========================== STEP LOOP ===========================
    CUR = ["init"]

    def R(role):
        CUR[0] = role

    def mm(*a, **k):
        _reg("Matmult", CUR[0])
        return _tag_role(nc.tensor.matmul(*a, **k), CUR[0])

    def sact(*a, **k):
        _reg("Activation", CUR[0])
        return _tag_role(nc.scalar.activation(*a, **k), CUR[0])

    def vtt(*a, **k):
        _reg("TensorTensor", CUR[0])
        return _tag_role(nc.vector.tensor_tensor(*a, **k), CUR[0])

    def g_tiles(pool, pfx):
        return {
            "ifo": pool.tile([P, 3 * H], F32, name=pfx + "ifo", tag=pfx + "ifo"),
            "g": pool.tile([P, H], F32, name=pfx + "g", tag=pfx + "g"),
        }

    def bkey(c):
        # PSUM zero-region bookkeeping key: one per 2KB bank
        if c < 8:
            return f"ifo{c // 4}"
        return "g" if c < 12 else "ifo2"

    def dst_of(g, c):
        if c < 8:
            return g["ifo"][:, c * P : (c + 1) * P]
        if c < 12:
            return g["g"][:, (c - 8) * P : (c - 7) * P]
        return g["ifo"][:, (c - 4) * P : (c - 3) * P]

    def hid_mms(g, c, W8v, Wg, h8v, hb, started, stop=False):
        """Hidden-side matmuls for one gate chunk: fp8 DR for i/f/o, bf16
        for g. One PSUM start per tile (zero region covers the bank)."""
        dst = dst_of(g, c)
        ty = bkey(c)
        if c in G_CHUNKS:
            j0 = (c - 8) * P
            for k in range(KC):
                mm(
                    dst, Wg[:, k * H + j0 : k * H + j0 + P],
                    hb[:, k * P : (k + 1) * P],
                    start=not started.get(ty, False),
                    stop=(stop and k == KC - 1),
                )
                started[ty] = True
        else:
            for k2 in range(2):
                mm(
                    dst, W8v[:, 2 * k2 : 2 * k2 + 2, c * P : (c + 1) * P],
                    h8v[:, 2 * k2 : 2 * k2 + 2, :],
                    start=not started.get(ty, False),
                    stop=(stop and k2 == 1),
                    perf_mode=DRM,
                )
                started[ty] = True

    def emit_A(g0, h08, h0b, chunks, started):
        """L0 hidden matmuls (start-side of each tile's group)."""
        R("A")
        h8v = h08.rearrange("p (k b) -> p k b", k=KC)
        for c in chunks:
            hid_mms(g0, c, W8h0v, Wg0, h8v, h0b, started)

    def emit_B(g0, t, parts=None):
        """L0 input matmuls, K=10 bf16 (exo + b0-ones + feed-const row).
        With parts: carries the group stops (t=0 only)."""
        rhs = inT[:, t * P : (t + 1) * P]
        if parts is None:
            R("B")
            for c in CH_ORDER:
                mm(dst_of(g0, c), wa0_sb[:, c * P : (c + 1) * P], rhs,
                   start=False, stop=False)
        else:
            for grp, part in zip(STOP_GROUPS, parts):
                R("B")
                for c in grp:
                    mm(dst_of(g0, c), wa0_sb[:, c * P : (c + 1) * P], rhs,
                       start=False, stop=(c in TILE_LAST))
                part()

    def emit_feed(g0, h18, parts):
        """W_feed @ h1 (fp8 DR, all 16 chunks): the tile-closing stops;
        tail parts interleave between stop groups."""
        h8v = h18.rearrange("p (k b) -> p k b", k=KC)
        for grp, part in zip(STOP_GROUPS, parts):
            R("feed")
            for c in grp:
                dst = dst_of(g0, c)
                for k2 in range(2):
                    mm(
                        dst, Wf8v[:, 2 * k2 : 2 * k2 + 2, c * P : (c + 1) * P],
                        h8v[:, 2 * k2 : 2 * k2 + 2, :],
                        start=False, stop=(c in TILE_LAST and k2 == 1),
                        perf_mode=DRM,
                    )
            part()

    def emit_C(g1, h18, h1b, chunks, started):
        """L1 h1-part matmuls + fp8 DR bias (start-side)."""
        h8v = h18.rearrange("p (k b) -> p k b", k=KC)
        for c in chunks:
            R("C")
            hid_mms(g1, c, W8h1v, Wgh1, h8v, h1b, started)
            R("Cb")
            mm(
                dst_of(g1, c), b1v[:, :, c * P : (c + 1) * P], on8v[:],
                start=False, stop=False, perf_mode=DRM,
            )

    def emit_D(g1, h08, h0b, parts):
        """L1 h0-part matmuls: the tile-closing stops; tail parts
        interleave between stop groups."""
        h8v = h08.rearrange("p (k b) -> p k b", k=KC)
        for grp, part in zip(STOP_GROUPS, parts):
            R("D")
            for c in grp:
                hid_mms(g1, c, W8i1v, Wgi1, h8v, h0b, {bkey(c): True},
                        stop=(c in TILE_LAST))
            part()

    def make_tail(g, c_prev, htag, ctag, h8_first):
        """Tail for one layer as 2 emit-callbacks matching stop groups
        (g, ifo). One 1536-wide sigmoid covers i/f/o; the DVE c-chain and
        tanh(c)/h-writes run in halves so downstream consumers start early.
        h8_first orders the fp8 h write before the bf16 one (L1: feed needs
        h18 first; L0: D-g needs h0 bf16 first)."""
        sg = act.tile([P, 3 * H], F16, name="sg", tag="sg")
        tg = act.tile([P, H], F16, name="tg", tag="tg")
        t1 = act.tile([P, H], F16, name="t1", tag="t1")
        t2 = act.tile([P, H], F16, name="t2", tag="t2")
        thc = act.tile([P, H], F16, name="thc", tag="thc")
        c_n = state.tile([P, H], F16, name=ctag, tag=ctag)
        h_b = state.tile([P, H], BF16, name=htag + "b", tag=htag + "b")
        h_8 = state.tile([P, H], F8, name=htag + "8", tag=htag + "8")
        sg_i, sg_f, sg_o = sg[:, 0:H], sg[:, H : 2 * H], sg[:, 2 * H : 3 * H]

        ISC = 1.0 / SCL
        HH = H // 2
        ha, hb = slice(0, HH), slice(HH, H)

        def p_g():
            R(htag + ":tg")
            sact(tg[:], g["g"][:], AF.Tanh, scale=ISC)

        def p_ifo():
            R(htag + ":sg")
            sact(sg[:], g["ifo"][:], AF.Sigmoid, scale=ISC)
            R(htag + ":cchain")
            vtt(t2[:, ha], sg_f[:, ha], c_prev[:, ha], op=OP.mult)
            vtt(t1[:, ha], sg_i[:, ha], tg[:, ha], op=OP.mult)
            vtt(c_n[:, ha], t1[:, ha], t2[:, ha], op=OP.add)
            sact(thc[:, ha], c_n[:, ha], AF.Tanh)
            vtt(t2[:, hb], sg_f[:, hb], c_prev[:, hb], op=OP.mult)
            vtt(t1[:, hb], sg_i[:, hb], tg[:, hb], op=OP.mult)
            vtt(c_n[:, hb], t1[:, hb], t2[:, hb], op=OP.add)
            sact(thc[:, hb], c_n[:, hb], AF.Tanh)
            R(htag + ":hwr")
            outs = [h_8, h_b] if h8_first else [h_b, h_8]
            for dst in outs:
                vtt(dst[:, ha], sg_o[:, ha], thc[:, ha], op=OP.mult)
                vtt(dst[:, hb], sg_o[:, hb], thc[:, hb], op=OP.mult)

        return [p_g, p_ifo], c_n, h_b, h_8

    # prologue: step-0 L0 gates (+interleaved tail via B stops) and L1 start
    g0_cur = g_tiles(g0p, "g0")
    emit_A(g0_cur, h08_c, h0b_c, CH_ORDER, {})
    parts0, c0_n, h0b_cur, h08_cur = make_tail(g0_cur, c0_c, "h0", "c0", h8_first=False)
    emit_B(g0_cur, 0, parts0)
    c0_c = c0_n
    g1_cur = g_tiles(g1p, "g1")
    emit_C(g1_cur, h18_c, h1b_c, CH_ORDER, {})

    for t in range(S):
        # ---- PE: L1 h0-part for t, interleaved with the L1 tail ----
        parts1, c1_n, h1b_n, h18_n = make_tail(g1_cur, c1_c, "h1", "c1", h8_first=True)
        emit_D(g1_cur, h08_cur, h0b_cur, parts1)
        c1_c = c1_n
        # ---- PE: L0 gates for t+1, first part (covers L1-tail latency) ----
        if t + 1 < S:
            g0_nxt = g_tiles(g0p, "g0")
            a_started = {}
            emit_A(g0_nxt, h08_cur, h0b_cur, CH_ORDER[:NA1], a_started)
        # ---- PE: projection yT(t) = Wp @ h1(t) (output only, off-path) ----
        pj = g1_cur["ifo"][0:1, 11 * P : 12 * P]
        R("proj")
        for k in range(2):
            mm(pj, wpT_sb[:, k : k + 1], h1b_n[:, k * P : (k + 1) * P],
               start=(k == 0), stop=False)
        if t + 1 < S:
            emit_A(g0_nxt, h08_cur, h0b_cur, CH_ORDER[NA1 : NA1 + 1], a_started)
        R("proj")
        for k in range(2, KC):
            mm(pj, wpT_sb[:, k : k + 1], h1b_n[:, k * P : (k + 1) * P],
               start=False, stop=(k == KC - 1))
        yo_t = yo.tile([1, P], F32, name="yout", tag="yout")
        R("y_out")
        vtt(yo_t[:], pj, bp_row[:], op=OP.add)
        dma(out_d[t : t + 1, :], yo_t[:])
        if t + 1 < S:
            emit_A(g0_nxt, h08_cur, h0b_cur, CH_ORDER[NA1 + 1 :], a_started)
            emit_B(g0_nxt, t + 1)
            parts0, c0_n, h0b_nxt, h08_nxt = make_tail(g0_nxt, c0_c, "h0", "c0", h8_first=False)
            emit_feed(g0_nxt, h18_n, parts0)
            c0_c, h0b_cur, h08_cur = c0_n, h0b_nxt, h08_nxt
            g1_nxt = g_tiles(g1p, "g1")
            emit_C(g1_nxt, h18_n, h1b_n, CH_ORDER, {})
            g0_cur, g1_cur = g0_nxt, g1_nxt


def make_nc(steps: int = S):
    nc = bacc.Bacc("TRN2", target_bir_lowering=False, debug=False)
    build_kernel(nc, steps)
    nc.compile()
    return nc


# ======================= host-side prep =======================

def _bf16(x):
    import ml_dtypes
    return np.ascontiguousarray(np.asarray(x, np.float32).astype(ml_dtypes.bfloat16))


def _f8(x):
    import ml_dtypes
    return np.ascontiguousarray(np.asarray(x, np.float32).astype(ml_dtypes.float8_e4m3fn))


def _f32(x):
    return np.ascontiguousarray(np.asarray(x, dtype=np.float32))


def _wT(W):
    """[G', H] f32 -> [128, KC*G'] with [p, k*G'+g] = W[g, k*128+p]."""
    W = np.asarray(W, dtype=np.float32)
    Gp = W.shape[0]
    return W.T.reshape(KC, P, Gp).transpose(1, 0, 2).reshape(P, KC * Gp)


def _hT4(h):
    """[B_loc, H] -> [128, H] with [p, k*128+b] = h[b, k*128+p]."""
    return h.T.reshape(KC, P, P).transpose(1, 0, 2).reshape(P, H)


def shard_inputs(inputs, steps: int):
    import ml_dtypes
    B = inputs["y0"].shape[0]
    nb = B // P
    Wih0 = np.asarray(inputs["W_ih0"], np.float32)
    Wp = np.asarray(inputs["W_proj"], np.float32)
    b0 = _f32(inputs["b_ih0"]) + _f32(inputs["b_hh0"])
    b1 = (_f32(inputs["b_ih1"]) + _f32(inputs["b_hh1"])) * SCL
    b1hi = b1.astype(ml_dtypes.float8_e4m3fn)
    b1lo = (b1 - b1hi.astype(np.float32)).astype(ml_dtypes.float8_e4m3fn)
    wa0 = np.concatenate(
        [
            Wih0[:, 1:9].T * SCL,          # exo rows
            b0.reshape(1, G) * SCL,        # ones-row bias
            Wih0[:, 0:1].T * SCL,          # feed-const row (wcol)
        ],
        axis=0,
    )
    Wf = np.outer(Wih0[:, 0], Wp[0])       # [G, H] y-feedback weights

    def wsplit(W):
        W = np.asarray(W, np.float32) * SCL
        return _f8(_wT(W)), _bf16(_wT(W[1024:1536]))

    w8h0, wg0 = wsplit(inputs["W_hh0"])
    w8i1, wgi1 = wsplit(inputs["W_ih1"])
    w8h1, wgh1 = wsplit(inputs["W_hh1"])
    rep = {
        "w8h0": w8h0, "wg0": wg0,
        "w8i1": w8i1, "wgi1": wgi1,
        "w8h1": w8h1, "wgh1": wgh1,
        "wf8": _f8(_wT(Wf * SCL)),
        "wa0": _bf16(wa0),
        "b1p": np.ascontiguousarray(
            np.stack([b1hi, b1lo]).reshape(1, 2 * G)),
        "wpT": _f32(Wp).reshape(KC, P).T.copy(),
        "bp": _f32(inputs["b_proj"]).reshape(1, 1),
        "wzT": _f32(inputs["W_z"]).T.copy(),
        "bz8": _f32(inputs["b_z"]).reshape(9, 1)[1:9].copy(),
        "bz0": _f32(inputs["b_z"]).reshape(9, 1)[0:1].copy(),
        "ones_row": _bf16(np.ones((1, S * P), np.float32)),
    }
    maps = []
    for i in range(nb):
        s = slice(i * P, (i + 1) * P)
        x = np.asarray(inputs["x_future"], np.float32)[s, :steps]  # [P, S, E]
        h0b = _bf16(_hT4(np.asarray(inputs["h0"], np.float32)[0, s]))
        h1b = _bf16(_hT4(np.asarray(inputs["h0"], np.float32)[1, s]))
        m = dict(rep)
        m.update(
            {
                "zT": _f32(inputs["z"][s]).T.copy(),
                "y0T": _f32(inputs["y0"][s]).reshape(1, P).copy(),
                "xfT": _bf16(x.transpose(2, 1, 0).reshape(EXO, steps * P)),
                "h0b": h0b,
                "h1b": h1b,
                "h08": _f8(h0b.astype(np.float32)),
                "h18": _f8(h1b.astype(np.float32)),
                "c0T": np.ascontiguousarray(
                    _hT4(np.asarray(inputs["c0"], np.float32)[0, s]).astype(np.float16)),
                "c1T": np.ascontiguousarray(
                    _hT4(np.asarray(inputs["c0"], np.float32)[1, s]).astype(np.float16)),
            }
        )
        maps.append(m)
    return maps


def assemble_output(results, steps: int):
    outs = [
        np.ascontiguousarray(np.asarray(rm["out"]).T).reshape(P, steps, 1)
        for rm in results
    ]
    return np.concatenate(outs, axis=0)


# ======================= public entry point =======================
_NC_CACHE = {}


def _get_nc():
    if "nc" not in _NC_CACHE:
        _NC_CACHE["nc"] = make_nc(S)
    return _NC_CACHE["nc"]


STEPS = S
N_CORES = 8


def kernel(**inputs):
    """Full-input entry point: shards batch over 8 NeuronCores, runs the
    Bass LSTM-decoder kernel, reassembles [B, steps, 1] float32 output."""
    from concourse.bass_utils import run_bass_kernel_spmd

    steps = int(inputs.get("steps", STEPS))
    assert steps == STEPS, f"kernel compiled for {STEPS} steps, got {steps}"
    nc = _get_nc()
    maps = shard_inputs(inputs, STEPS)
    res = run_bass_kernel_spmd(nc, maps, list(range(N_CORES)))
    return assemble_output(res.results, STEPS).astype(np.float32)
